# revision 18
# baseline (speedup 1.0000x reference)
"""Trainium2 Bass kernel for nn_EnhancedUberCRSN (scatter_memory).

Pure data parallel over the batch dim B=16384 across 8 NeuronCores.
Per core: 2048 samples, processed as 16 tiles of 128 samples.

Math (per sample b):
  g = sigmoid(ctrl); push/pop/stay = g / (sum(g)+EPS)
  new_ptr = push*roll(ptr,1) + pop*roll(ptr,-1) + stay*ptr
  m = mem*(1-push) + push*z_flat                       (z_flat = [z_re, z_im])
  scores = (m @ Cq)(m @ Ck).T / 8 = m @ A @ m.T        (A = Cq Ck^T / 8)
  attn = softmax(scores); att = attn @ (m @ Cv)
  z_read = sum_s new_ptr[s] * att[s]
  active = mean_b count_s(new_ptr > 0.1)

Device layout: samples on partitions for elementwise ("sample-major"),
features on partitions for matmuls ("feature-major", via PE transposes).
Attention runs in groups of 8 samples (8 x 16 slots = 128 partition rows).
"""

import os
import sys
from contextlib import ExitStack

import numpy as np

sys.path.insert(0, "/opt/trn_rl_repo")

import concourse.bass as bass  # noqa: E402
import concourse.tile as tile  # noqa: E402
from concourse import bacc, mybir  # noqa: E402

EPS = 1e-6
B_FULL = 16384
N_CORES = 8
B_CORE = B_FULL // N_CORES
S = 16
D = 64
F = 2 * D  # 128 features (real|imag)
P = 128  # partitions / tile of samples
G = 8  # samples per attention group
NEG_INIT = -3.0e38

f32 = mybir.dt.float32
f32r = mybir.dt.float32r
bf16 = mybir.dt.bfloat16
AF = mybir.ActivationFunctionType
ALU = mybir.AluOpType


# ----------------------------------------------------------------------------
# device program
# ----------------------------------------------------------------------------


def build_program(b_core: int):
    """Build the per-core Bass program. Returns (nc, names dict)."""
    n_tiles = b_core // P
    assert n_tiles * P == b_core

    nc = bacc.Bacc(
        "TRN2",
        target_bir_lowering=False,
        debug=False,
        enable_asserts=False,
    )

    # DRAM I/O ---------------------------------------------------------------
    mem_h = nc.dram_tensor("mem", [b_core, S, F], f32, kind="ExternalInput").ap()
    zre_h = nc.dram_tensor("z_real", [b_core, D], f32, kind="ExternalInput").ap()
    zim_h = nc.dram_tensor("z_imag", [b_core, D], f32, kind="ExternalInput").ap()
    ptr_h = nc.dram_tensor("ptr", [b_core, S], f32, kind="ExternalInput").ap()
    ctrl_h = nc.dram_tensor("ctrl", [b_core, 3], f32, kind="ExternalInput").ap()
    A_h = nc.dram_tensor("Amat", [F, F], f32r, kind="ExternalInput").ap()
    Cv_h = nc.dram_tensor("Cv", [F, F], f32r, kind="ExternalInput").ap()
    ident_h = nc.dram_tensor("ident", [P, P], f32, kind="ExternalInput").ap()
    onesb_h = nc.dram_tensor("onesblk", [P, G], bf16, kind="ExternalInput").ap()
    maskp_h = nc.dram_tensor("maskP", [P, P], f32, kind="ExternalInput").ap()
    mneg_h = nc.dram_tensor("mneg1", [1, P], bf16, kind="ExternalInput").ap()
    mone_h = nc.dram_tensor("mone1", [1, 2 * P], bf16, kind="ExternalInput").ap()
    blkl_h = nc.dram_tensor("blkL", [G, P], bf16, kind="ExternalInput").ap()
    blkre_h = nc.dram_tensor("blkRe", [G, 2 * P], bf16, kind="ExternalInput").ap()
    blkro_h = nc.dram_tensor("blkRo", [G, 2 * P], bf16, kind="ExternalInput").ap()

    memo_h = nc.dram_tensor("mem_new", [b_core, S, F], f32, kind="ExternalOutput").ap()
    nptr_h = nc.dram_tensor("new_ptr", [b_core, S], f32, kind="ExternalOutput").ap()
    zrr_h = nc.dram_tensor("z_read_real", [b_core, D], f32, kind="ExternalOutput").ap()
    zri_h = nc.dram_tensor("z_read_imag", [b_core, D], f32, kind="ExternalOutput").ap()
    act_h = nc.dram_tensor("active_part", [1, 1], f32, kind="ExternalOutput").ap()

    with tile.TileContext(nc) as tc, ExitStack() as ctx:
        ep = ctx.enter_context  # shorthand

        consts = ep(tc.tile_pool(name="consts", bufs=1))
        io = ep(tc.tile_pool(name="io", bufs=3))
        big = ep(tc.tile_pool(name="bigbuf", bufs=2))
        sm = ep(tc.tile_pool(name="small", bufs=2))
        grp = ep(tc.tile_pool(name="grp", bufs=3))
        acc_pool = ep(tc.tile_pool(name="accp", bufs=1))
        ps_big = ep(tc.tile_pool(name="psbig", bufs=2, space="PSUM"))
        ps_sc = ep(tc.tile_pool(name="pssc", bufs=4, space="PSUM"))
        ps_zrp = ep(tc.tile_pool(name="pszr", bufs=2, space="PSUM"))

        # constants, loaded once
        A_t = consts.tile([F, F], f32r)
        nc.sync.dma_start(A_t[:], A_h[:])
        Cv_t = consts.tile([F, F], f32r)
        nc.sync.dma_start(Cv_t[:], Cv_h[:])
        ident_t = consts.tile([P, P], f32)
        nc.sync.dma_start(ident_t[:], ident_h[:])
        onesb_t = consts.tile([P, G], bf16)
        nc.sync.dma_start(onesb_t[:], onesb_h[:])
        maskp_t = consts.tile([P, P], f32)
        nc.sync.dma_start(maskp_t[:], maskp_h[:])
        mneg_t = consts.tile([1, P], bf16)
        nc.sync.dma_start(mneg_t[:], mneg_h[:])
        mone_t = consts.tile([1, 2 * P], bf16)
        nc.sync.dma_start(mone_t[:], mone_h[:])
        blkl_t = consts.tile([G, P], bf16)
        nc.sync.dma_start(blkl_t[:], blkl_h[:])
        blkre_t = consts.tile([G, 2 * P], bf16)
        nc.sync.dma_start(blkre_t[:], blkre_h[:])
        blkro_t = consts.tile([G, 2 * P], bf16)
        nc.sync.dma_start(blkro_t[:], blkro_h[:])
        ones_t = consts.tile([P, 1], f32)
        nc.vector.memset(ones_t[:], 1.0)

        acc_t = acc_pool.tile([P, 1], f32)  # active-slot count accumulator
        nc.vector.memset(acc_t[:], 0.0)

        for t in range(n_tiles):
            rows = slice(t * P, (t + 1) * P)

            # ---- loads ----
            mem_t = io.tile([P, S * F], f32, tag="mem")
            nc.sync.dma_start(mem_t[:], mem_h[rows].rearrange("b s f -> b (s f)"))
            zf = io.tile([P, F], f32, tag="zf")
            nc.sync.dma_start(zf[:, 0:D], zre_h[rows])
            nc.sync.dma_start(zf[:, D:F], zim_h[rows])
            ptr_t = io.tile([P, S], f32, tag="ptr")
            nc.sync.dma_start(ptr_t[:], ptr_h[rows])
            ctrl_t = io.tile([P, 3], f32, tag="ctrl")
            nc.sync.dma_start(ctrl_t[:], ctrl_h[rows])

            # ---- gates ----
            sg = sm.tile([P, 3], f32, tag="sg")
            nc.scalar.activation(sg[:], ctrl_t[:], AF.Exp, scale=-1.0)
            nc.vector.tensor_scalar_add(sg[:], sg[:], 1.0)
            nc.vector.reciprocal(sg[:], sg[:])
            tot = sm.tile([P, 1], f32, tag="tot")
            nc.vector.tensor_reduce(tot[:], sg[:], axis=mybir.AxisListType.X, op=ALU.add)
            rt = sm.tile([P, 1], f32, tag="rt")
            nc.vector.tensor_scalar_add(tot[:], tot[:], EPS)
            nc.vector.reciprocal(rt[:], tot[:])
            push = sm.tile([P, 1], f32, tag="push")
            pop = sm.tile([P, 1], f32, tag="pop")
            stay = sm.tile([P, 1], f32, tag="stay")
            nc.gpsimd.tensor_tensor(push[:], sg[:, 0:1], rt[:], op=ALU.mult)
            nc.gpsimd.tensor_tensor(pop[:], sg[:, 1:2], rt[:], op=ALU.mult)
            nc.gpsimd.tensor_tensor(stay[:], sg[:, 2:3], rt[:], op=ALU.mult)
            onemp = sm.tile([P, 1], f32, tag="onemp")
            nc.gpsimd.tensor_scalar(onemp[:], push[:], -1.0, 1.0, op0=ALU.mult, op1=ALU.add)
            pz = sm.tile([P, F], f32, tag="pz")
            nc.gpsimd.tensor_scalar_mul(pz[:], zf[:], push[:])

            # ---- mem_new = mem*(1-push) + push*z  (broadcast z over slots) ----
            mn = big.tile([P, S * F], f32, tag="mn")
            nc.vector.scalar_tensor_tensor(
                mn[:].rearrange("p (s f) -> p s f", f=F),
                mem_t[:].rearrange("p (s f) -> p s f", f=F),
                onemp[:],
                pz[:].unsqueeze(1).broadcast_to([P, S, F]),
                op0=ALU.mult,
                op1=ALU.add,
            )
            nc.sync.dma_start(memo_h[rows].rearrange("b s f -> b (s f)"), mn[:])

            # ---- new_ptr ----
            t_up = sm.tile([P, S], f32, tag="t_up")
            nc.gpsimd.tensor_scalar_mul(t_up[:, 1:S], ptr_t[:, 0 : S - 1], push[:])
            nc.gpsimd.tensor_scalar_mul(t_up[:, 0:1], ptr_t[:, S - 1 : S], push[:])
            t_dn = sm.tile([P, S], f32, tag="t_dn")
            nc.gpsimd.tensor_scalar_mul(t_dn[:, 0 : S - 1], ptr_t[:, 1:S], pop[:])
            nc.gpsimd.tensor_scalar_mul(t_dn[:, S - 1 : S], ptr_t[:, 0:1], pop[:])
            npt = sm.tile([P, S], f32, tag="npt")
            nc.gpsimd.tensor_scalar_mul(npt[:], ptr_t[:], stay[:])
            nc.gpsimd.tensor_add(npt[:], npt[:], t_up[:])
            nc.gpsimd.tensor_add(npt[:], npt[:], t_dn[:])
            nc.sync.dma_start(nptr_h[rows], npt[:])

            # active slots partial: acc += sum_s (new_ptr > 0.1)
            acg = sm.tile([P, S], f32, tag="acg")
            acs = sm.tile([P, 1], f32, tag="acs")
            nc.vector.tensor_scalar(
                acg[:], npt[:], 0.1, None, op0=ALU.is_gt, op1=ALU.add, accum_out=acs[:]
            )
            nc.gpsimd.tensor_add(acc_t[:], acc_t[:], acs[:])

            # ---- ptr_cols: new_ptr rearranged to [(b,s), group] columns ----
            # E[b', (b,s)] = npt[b', s];  Pm = E.T @ I -> Pm[(b,s), b'] = npt[b', s]
            E_t = sm.tile([P, P], f32, tag="E_t")
            nc.gpsimd.tensor_copy(
                E_t[:].rearrange("p (b s) -> p b s", s=S),
                npt[:].unsqueeze(1).broadcast_to([P, G, S]),
            )
            psP = ps_zrp.tile([P, P], f32, tag="zrq")
            nc.tensor.matmul(psP[:], E_t[:], ident_t[:])
            mPm = sm.tile([P, P], f32, tag="mPm")
            nc.vector.tensor_tensor(mPm[:], psP[:], maskp_t[:], op=ALU.mult)
            ptr_cols = sm.tile([P, S], f32, tag="ptr_cols")
            nc.vector.tensor_reduce(
                ptr_cols[:],
                mPm[:].rearrange("p (g j) -> p g j", j=G),
                axis=mybir.AxisListType.X,
                op=ALU.add,
            )

            # ---- transposes (mT), W1T = A.T @ mT, VT = Cv.T @ mT ----
            mTt = big.tile([P, S * F], f32r, tag="mT")
            w1T = big.tile([P, S * F], f32r, tag="w1T")
            vta = big.tile([P, S * F], bf16, tag="vta")
            for sb in range(4):
                cols = slice(512 * sb, 512 * (sb + 1))
                psT = ps_big.tile([P, 512], f32, tag="ps512")
                for j in range(4):
                    nc.tensor.matmul(
                        psT[:, 128 * j : 128 * (j + 1)],
                        mn[:, 512 * sb + 128 * j : 512 * sb + 128 * (j + 1)],
                        ident_t[:],
                        is_transpose=True,
                        start=(j == 0),
                        stop=(j == 3),
                    )
                # scatter into b-outer layout: col = 16*b + s  (s = 4*sb + j)
                mT_dst = mTt[:].rearrange("p (b s) -> p s b", s=S)[
                    :, 4 * sb : 4 * (sb + 1), :
                ]
                if sb % 2 == 0:
                    nc.vector.tensor_copy(mT_dst, psT[:])
                else:
                    nc.scalar.copy(mT_dst, psT[:])

            for sb in range(4):
                cols = slice(512 * sb, 512 * (sb + 1))
                psW = ps_big.tile([P, 512], f32, tag="ps512")
                nc.tensor.matmul(
                    psW[:], A_t[:], mTt[:, cols]
                )
                if sb % 2 == 0:
                    nc.scalar.copy(w1T[:, cols], psW[:])
                else:
                    nc.vector.tensor_copy(w1T[:, cols], psW[:])

                psV = ps_big.tile([P, 512], f32, tag="ps512")
                nc.tensor.matmul(
                    psV[:], Cv_t[:], mTt[:, cols]
                )
                nc.vector.tensor_copy(vta[:, cols], psV[:])

            # ---- scores + softmax per group of 8 samples ----
            attn_all = big.tile([P, S * F], bf16, tag="attn_all")
            vsm_all = big.tile([P, S * F], bf16, tag="vsm_all")
            nc.sync.dma_start_transpose(
                vsm_all[:].rearrange("p (g q) -> p g q", q=P), vta[:]
            )
            negmax = sm.tile([P, S], f32, tag="negmax")
            rsums = sm.tile([P, S], f32, tag="rsums")

            for g in range(S):
                k = g // 2
                half = g % 2
                blkr_t = blkre_t if half == 0 else blkro_t
                ps_sc_t = ps_sc.tile([P, 256], f32, tag="sc")
                # mask first (-1e30 everywhere, +1e30 on own diag blocks),
                # then accumulate scores: off-diag stays -1e30, diag exact.
                nc.tensor.matmul(
                    ps_sc_t[:], mneg_t[:], mone_t[:], start=True, stop=False
                )
                nc.tensor.matmul(
                    ps_sc_t[:], blkl_t[:], blkr_t[:], start=False, stop=False
                )
                nc.tensor.matmul(
                    ps_sc_t[:],
                    w1T[:, 128 * g : 128 * (g + 1)],
                    mTt[:, 256 * k : 256 * (k + 1)],
                    start=False,
                    stop=True,
                )
                half_ap = ps_sc_t[:, 128 * half : 128 * (half + 1)]
                nc.vector.tensor_reduce(
                    negmax[:, g : g + 1], half_ap,
                    axis=mybir.AxisListType.X, op=ALU.max, negate=True,
                )
                attn_g = attn_all[:, 128 * g : 128 * (g + 1)]
                nc.scalar.activation(
                    attn_g,
                    half_ap,
                    AF.Exp,
                    bias=negmax[:, g : g + 1],
                    scale=1.0,
                    accum_out=rsums[:, g : g + 1],
                )

            recip = sm.tile([P, S], f32, tag="recip")
            nc.vector.reciprocal(recip[:], rsums[:])
            wS = sm.tile([P, S], f32, tag="wS")
            nc.vector.tensor_tensor(wS[:], recip[:], ptr_cols[:], op=ALU.mult)

            # ---- read, folded through attention ----
            # z_read_g = W8_g^T attn_g vsm_g, W8_g = onesblk * wS[:, g]
            # Y_g = attn_g^T @ W8_g  [ (b,t), j ];  z_read_g = Y_g^T @ vsm_g [j, o]
            zr = sm.tile([G, S * F], f32, tag="zr")
            W8a = sm.tile([P, G * S], bf16, tag="w8")
            for g in range(S):
                nc.gpsimd.tensor_scalar_mul(
                    W8a[:, G * g : G * (g + 1)], onesb_t[:], wS[:, g : g + 1]
                )
            for g in range(S):
                q = g % 4
                if q == 0:
                    psY4 = ps_sc.tile([P, 4 * G], f32, tag="sc")
                nc.tensor.matmul(
                    psY4[:, G * q : G * (q + 1)],
                    attn_all[:, 128 * g : 128 * (g + 1)],
                    W8a[:, G * g : G * (g + 1)],
                    start=(q == 0),
                    stop=(q == 3),
                )
                if q == 3:
                    Ysb4 = grp.tile([P, 4 * G], bf16, tag="Ysb")
                    nc.scalar.copy(Ysb4[:], psY4[:])
                    for q2 in range(4):
                        g2 = (g // 4) * 4 + q2
                        if q2 == 0:
                            ps_zr = ps_zrp.tile([G, 512], f32, tag="zrq")
                        nc.tensor.matmul(
                            ps_zr[:, 128 * q2 : 128 * (q2 + 1)],
                            Ysb4[:, G * q2 : G * (q2 + 1)],
                            vsm_all[:, 128 * g2 : 128 * (g2 + 1)],
                            start=(q2 == 0),
                            stop=(q2 == 3),
                        )
                    c = g // 4
                    nc.scalar.copy(zr[:, 512 * c : 512 * (c + 1)], ps_zr[:])
            zr3 = zr[:].rearrange("p (g o) -> p g o", o=F)
            nc.sync.dma_start(
                zrr_h[rows].rearrange("(g b) o -> b g o", b=G), zr3[:, :, 0:D]
            )
            nc.sync.dma_start(
                zri_h[rows].rearrange("(g b) o -> b g o", b=G), zr3[:, :, D:F]
            )

        # ---- active_slots: cross-partition sum of acc ----
        psA = ps_zrp.tile([1, 1], f32, tag="zrq")
        nc.tensor.matmul(psA[:], acc_t[:], ones_t[:])
        act_s = sm.tile([1, 1], f32, tag="act_s")
        nc.vector.tensor_copy(act_s[:], psA[:])
        nc.sync.dma_start(act_h[:], act_s[:])

    nc.compile()
    return nc


# ----------------------------------------------------------------------------
# host-side constants
# ----------------------------------------------------------------------------


def host_constants(w_q_real, w_q_imag, w_k_real, w_k_imag, w_v_real, w_v_imag):
    def cmat(wr, wi):
        wr = np.asarray(wr, np.float64)
        wi = np.asarray(wi, np.float64)
        top = np.concatenate([wr.T, wi.T], axis=1)
        bot = np.concatenate([-wi.T, wr.T], axis=1)
        return np.concatenate([top, bot], axis=0)  # [2D, 2D]

    Cq = cmat(w_q_real, w_q_imag)
    Ck = cmat(w_k_real, w_k_imag)
    Cv = cmat(w_v_real, w_v_imag)
    A = (Cq @ Ck.T) * (D ** -0.5)

    ident = np.eye(P, dtype=np.float32)
    onesblk = np.zeros((P, G), np.float32)
    for p_ in range(P):
        onesblk[p_, p_ // S] = 1.0
    maskP = np.zeros((P, P), np.float32)
    for p_ in range(P):
        b = p_ // S
        for col in range(P):
            if col % G == b:
                maskP[p_, col] = 1.0
    import ml_dtypes

    BIG = 1.0e30
    mneg1 = np.full((1, P), -BIG, np.float32)
    mone1 = np.ones((1, 2 * P), np.float32)
    blkL = np.zeros((G, P), np.float32)
    for j in range(G):
        for p_ in range(P):
            if p_ // S == j:
                blkL[j, p_] = BIG
    blkRe = np.zeros((G, 2 * P), np.float32)
    blkRo = np.zeros((G, 2 * P), np.float32)
    for j in range(G):
        for n in range(2 * P):
            bp = n // S  # sample index within the 16-sample pair
            if bp == j:
                blkRe[j, n] = 1.0
            if bp == G + j:
                blkRo[j, n] = 1.0

    return {
        "Amat": A.astype(np.float32),
        "Cv": Cv.astype(np.float32),
        "ident": ident,
        "onesblk": onesblk.astype(ml_dtypes.bfloat16),
        "maskP": maskP,
        "mneg1": mneg1.astype(ml_dtypes.bfloat16),
        "mone1": mone1.astype(ml_dtypes.bfloat16),
        "blkL": blkL.astype(ml_dtypes.bfloat16),
        "blkRe": blkRe.astype(ml_dtypes.bfloat16),
        "blkRo": blkRo.astype(ml_dtypes.bfloat16),
    }


def make_in_maps(inputs, b_core):
    consts = host_constants(
        inputs["w_q_real"], inputs["w_q_imag"],
        inputs["w_k_real"], inputs["w_k_imag"],
        inputs["w_v_real"], inputs["w_v_imag"],
    )
    n_cores = int(np.asarray(inputs["mem"]).shape[0]) // b_core
    in_maps = []
    for c in range(n_cores):
        rows = slice(c * b_core, (c + 1) * b_core)
        m = {
            "mem": np.ascontiguousarray(np.asarray(inputs["mem"], np.float32)[rows]),
            "z_real": np.ascontiguousarray(np.asarray(inputs["z_real"], np.float32)[rows]),
            "z_imag": np.ascontiguousarray(np.asarray(inputs["z_imag"], np.float32)[rows]),
            "ptr": np.ascontiguousarray(np.asarray(inputs["ptr"], np.float32)[rows]),
            "ctrl": np.ascontiguousarray(np.asarray(inputs["ctrl"], np.float32)[rows]),
        }
        m.update(consts)
        in_maps.append(m)
    return in_maps


_PROGRAM_CACHE = {}


def _get_program(b_core):
    if b_core not in _PROGRAM_CACHE:
        _PROGRAM_CACHE[b_core] = build_program(b_core)
    return _PROGRAM_CACHE[b_core]


def kernel(**inputs):
    B = int(np.asarray(inputs["z_real"]).shape[0])
    b_core = B // N_CORES
    nc = _get_program(b_core)
    in_maps = make_in_maps(inputs, b_core)

    from concourse.bass_utils import run_bass_kernel_spmd

    res = run_bass_kernel_spmd(nc, in_maps, list(range(N_CORES)))
    outs = res.results

    z_read_real = np.concatenate([o["z_read_real"] for o in outs], axis=0)
    z_read_imag = np.concatenate([o["z_read_imag"] for o in outs], axis=0)
    mem_new = np.concatenate([o["mem_new"] for o in outs], axis=0)
    new_ptr = np.concatenate([o["new_ptr"] for o in outs], axis=0)
    active = np.float32(sum(float(o["active_part"][0, 0]) for o in outs) / B)
    return (
        z_read_real.astype(np.float32),
        z_read_imag.astype(np.float32),
        mem_new.astype(np.float32),
        new_ptr.astype(np.float32),
        np.float32(active),
    )


# revision 26
# speedup vs baseline: 98.1293x; 98.1293x over previous
"""Trainium2 Bass kernel for nn_EnhancedUberCRSN (scatter_memory).

Pure data parallel over the batch dim B=16384 across 8 NeuronCores.
Per core: 2048 samples, processed as 16 tiles of 128 samples.

Math (per sample b):
  g = sigmoid(ctrl); push/pop/stay = g / (sum(g)+EPS)
  new_ptr = push*roll(ptr,1) + pop*roll(ptr,-1) + stay*ptr
  m = mem*(1-push) + push*z_flat                       (z_flat = [z_re, z_im])
  scores = (m @ Cq)(m @ Ck).T / 8 = m @ A @ m.T        (A = Cq Ck^T / 8)
  attn = softmax(scores); att = attn @ (m @ Cv)
  z_read = sum_s new_ptr[s] * att[s]
  active = mean_b count_s(new_ptr > 0.1)

Device layout: samples on partitions for elementwise ("sample-major"),
features on partitions for matmuls ("feature-major", via PE transposes).
Attention runs in groups of 8 samples (8 x 16 slots = 128 partition rows).
"""

import os
import sys
from contextlib import ExitStack

import numpy as np

sys.path.insert(0, "/opt/trn_rl_repo")

import concourse.bass as bass  # noqa: E402
import concourse.tile as tile  # noqa: E402
from concourse import bacc, mybir  # noqa: E402

EPS = 1e-6
B_FULL = 16384
N_CORES = 8
B_CORE = B_FULL // N_CORES
S = 16
D = 64
F = 2 * D  # 128 features (real|imag)
P = 128  # partitions / tile of samples
G = 8  # samples per attention group
NEG_INIT = -3.0e38

f32 = mybir.dt.float32
f32r = mybir.dt.float32r
bf16 = mybir.dt.bfloat16
AF = mybir.ActivationFunctionType
ALU = mybir.AluOpType


# ----------------------------------------------------------------------------
# device program
# ----------------------------------------------------------------------------


def build_program(b_core: int):
    """Build the per-core Bass program. Returns (nc, names dict)."""
    n_tiles = b_core // P
    assert n_tiles * P == b_core

    nc = bacc.Bacc(
        "TRN2",
        target_bir_lowering=False,
        debug=False,
        enable_asserts=False,
    )

    # DRAM I/O ---------------------------------------------------------------
    mem_h = nc.dram_tensor("mem", [b_core, S, F], f32, kind="ExternalInput").ap()
    zre_h = nc.dram_tensor("z_real", [b_core, D], f32, kind="ExternalInput").ap()
    zim_h = nc.dram_tensor("z_imag", [b_core, D], f32, kind="ExternalInput").ap()
    ptr_h = nc.dram_tensor("ptr", [b_core, S], f32, kind="ExternalInput").ap()
    ctrl_h = nc.dram_tensor("ctrl", [b_core, 3], f32, kind="ExternalInput").ap()
    A_h = nc.dram_tensor("Amat", [F, F], f32r, kind="ExternalInput").ap()
    Cv_h = nc.dram_tensor("Cv", [F, F], f32r, kind="ExternalInput").ap()
    ident_h = nc.dram_tensor("ident", [P, P], f32, kind="ExternalInput").ap()
    onesb_h = nc.dram_tensor("onesblk", [P, G], bf16, kind="ExternalInput").ap()
    maskp_h = nc.dram_tensor("maskP", [P, P], f32, kind="ExternalInput").ap()
    mneg_h = nc.dram_tensor("mneg1", [1, P], bf16, kind="ExternalInput").ap()
    mone_h = nc.dram_tensor("mone1", [1, 2 * P], bf16, kind="ExternalInput").ap()
    blkl_h = nc.dram_tensor("blkL", [G, P], bf16, kind="ExternalInput").ap()
    blkre_h = nc.dram_tensor("blkRe", [G, 2 * P], bf16, kind="ExternalInput").ap()
    blkro_h = nc.dram_tensor("blkRo", [G, 2 * P], bf16, kind="ExternalInput").ap()

    memo_h = nc.dram_tensor("mem_new", [b_core, S, F], f32, kind="ExternalOutput").ap()
    nptr_h = nc.dram_tensor("new_ptr", [b_core, S], f32, kind="ExternalOutput").ap()
    zrr_h = nc.dram_tensor("z_read_real", [b_core, D], f32, kind="ExternalOutput").ap()
    zri_h = nc.dram_tensor("z_read_imag", [b_core, D], f32, kind="ExternalOutput").ap()
    act_h = nc.dram_tensor("active_part", [1, 1], f32, kind="ExternalOutput").ap()

    with tile.TileContext(nc) as tc, ExitStack() as ctx:
        ep = ctx.enter_context  # shorthand

        consts = ep(tc.tile_pool(name="consts", bufs=1))
        io = ep(tc.tile_pool(name="io", bufs=3))
        big = ep(tc.tile_pool(name="bigbuf", bufs=2))
        sm = ep(tc.tile_pool(name="small", bufs=2))
        grp = ep(tc.tile_pool(name="grp", bufs=3))
        acc_pool = ep(tc.tile_pool(name="accp", bufs=1))
        ps_big = ep(tc.tile_pool(name="psbig", bufs=2, space="PSUM"))
        ps_sc = ep(tc.tile_pool(name="pssc", bufs=4, space="PSUM"))
        ps_zrp = ep(tc.tile_pool(name="pszr", bufs=2, space="PSUM"))

        # constants, loaded once
        A_t = consts.tile([F, F], f32r)
        nc.sync.dma_start(A_t[:], A_h[:])
        Cv_t = consts.tile([F, F], f32r)
        nc.sync.dma_start(Cv_t[:], Cv_h[:])
        ident_t = consts.tile([P, P], f32)
        nc.sync.dma_start(ident_t[:], ident_h[:])
        onesb_t = consts.tile([P, G], bf16)
        nc.sync.dma_start(onesb_t[:], onesb_h[:])
        maskp_t = consts.tile([P, P], f32)
        nc.sync.dma_start(maskp_t[:], maskp_h[:])
        mneg_t = consts.tile([1, P], bf16)
        nc.sync.dma_start(mneg_t[:], mneg_h[:])
        mone_t = consts.tile([1, 2 * P], bf16)
        nc.sync.dma_start(mone_t[:], mone_h[:])
        blkl_t = consts.tile([G, P], bf16)
        nc.sync.dma_start(blkl_t[:], blkl_h[:])
        blkre_t = consts.tile([G, 2 * P], bf16)
        nc.sync.dma_start(blkre_t[:], blkre_h[:])
        blkro_t = consts.tile([G, 2 * P], bf16)
        nc.sync.dma_start(blkro_t[:], blkro_h[:])
        ones_t = consts.tile([P, 1], f32)
        nc.vector.memset(ones_t[:], 1.0)

        acc_t = acc_pool.tile([P, 1], f32)  # active-slot count accumulator
        nc.vector.memset(acc_t[:], 0.0)

        for t in range(n_tiles):
            rows = slice(t * P, (t + 1) * P)

            # ---- loads ----
            mem_t = io.tile([P, S * F], f32, tag="mem")
            nc.sync.dma_start(mem_t[:], mem_h[rows].rearrange("b s f -> b (s f)"))
            zf = io.tile([P, F], f32, tag="zf")
            nc.sync.dma_start(zf[:, 0:D], zre_h[rows])
            nc.sync.dma_start(zf[:, D:F], zim_h[rows])
            ptr_t = io.tile([P, S], f32, tag="ptr")
            nc.sync.dma_start(ptr_t[:], ptr_h[rows])
            ctrl_t = io.tile([P, 3], f32, tag="ctrl")
            nc.sync.dma_start(ctrl_t[:], ctrl_h[rows])

            # ---- gates ----
            sg = sm.tile([P, 3], f32, tag="sg")
            nc.scalar.activation(sg[:], ctrl_t[:], AF.Exp, scale=-1.0)
            nc.vector.tensor_scalar_add(sg[:], sg[:], 1.0)
            nc.vector.reciprocal(sg[:], sg[:])
            tot = sm.tile([P, 1], f32, tag="tot")
            nc.vector.tensor_reduce(tot[:], sg[:], axis=mybir.AxisListType.X, op=ALU.add)
            rt = sm.tile([P, 1], f32, tag="rt")
            nc.vector.tensor_scalar_add(tot[:], tot[:], EPS)
            nc.vector.reciprocal(rt[:], tot[:])
            push = sm.tile([P, 1], f32, tag="push")
            pop = sm.tile([P, 1], f32, tag="pop")
            stay = sm.tile([P, 1], f32, tag="stay")
            nc.gpsimd.tensor_tensor(push[:], sg[:, 0:1], rt[:], op=ALU.mult)
            nc.gpsimd.tensor_tensor(pop[:], sg[:, 1:2], rt[:], op=ALU.mult)
            nc.gpsimd.tensor_tensor(stay[:], sg[:, 2:3], rt[:], op=ALU.mult)
            onemp = sm.tile([P, 1], f32, tag="onemp")
            nc.gpsimd.tensor_scalar(onemp[:], push[:], -1.0, 1.0, op0=ALU.mult, op1=ALU.add)
            pz = sm.tile([P, F], f32, tag="pz")
            nc.gpsimd.tensor_scalar_mul(pz[:], zf[:], push[:])

            # ---- mem_new = mem*(1-push) + push*z  (broadcast z over slots) ----
            mn = big.tile([P, S * F], f32, tag="mn")
            nc.vector.scalar_tensor_tensor(
                mn[:].rearrange("p (s f) -> p s f", f=F),
                mem_t[:].rearrange("p (s f) -> p s f", f=F),
                onemp[:],
                pz[:].unsqueeze(1).broadcast_to([P, S, F]),
                op0=ALU.mult,
                op1=ALU.add,
            )
            nc.sync.dma_start(memo_h[rows].rearrange("b s f -> b (s f)"), mn[:])

            # ---- new_ptr ----
            t_up = sm.tile([P, S], f32, tag="t_up")
            nc.gpsimd.tensor_scalar_mul(t_up[:, 1:S], ptr_t[:, 0 : S - 1], push[:])
            nc.gpsimd.tensor_scalar_mul(t_up[:, 0:1], ptr_t[:, S - 1 : S], push[:])
            t_dn = sm.tile([P, S], f32, tag="t_dn")
            nc.gpsimd.tensor_scalar_mul(t_dn[:, 0 : S - 1], ptr_t[:, 1:S], pop[:])
            nc.gpsimd.tensor_scalar_mul(t_dn[:, S - 1 : S], ptr_t[:, 0:1], pop[:])
            npt = sm.tile([P, S], f32, tag="npt")
            nc.gpsimd.tensor_scalar_mul(npt[:], ptr_t[:], stay[:])
            nc.gpsimd.tensor_add(npt[:], npt[:], t_up[:])
            nc.gpsimd.tensor_add(npt[:], npt[:], t_dn[:])
            nc.sync.dma_start(nptr_h[rows], npt[:])

            # active slots partial: acc += sum_s (new_ptr > 0.1)
            acg = sm.tile([P, S], f32, tag="acg")
            acs = sm.tile([P, 1], f32, tag="acs")
            nc.vector.tensor_scalar(
                acg[:], npt[:], 0.1, None, op0=ALU.is_gt, op1=ALU.add, accum_out=acs[:]
            )
            nc.gpsimd.tensor_add(acc_t[:], acc_t[:], acs[:])

            # ---- ptr_cols: new_ptr rearranged to [(b,s), group] columns ----
            # E[b', (b,s)] = npt[b', s];  Pm = E.T @ I -> Pm[(b,s), b'] = npt[b', s]
            E_t = sm.tile([P, P], f32, tag="E_t")
            nc.gpsimd.tensor_copy(
                E_t[:].rearrange("p (b s) -> p b s", s=S),
                npt[:].unsqueeze(1).broadcast_to([P, G, S]),
            )
            psP = ps_zrp.tile([P, P], f32, tag="zrq")
            nc.tensor.matmul(psP[:], E_t[:], ident_t[:])
            mPm = sm.tile([P, P], f32, tag="mPm")
            nc.vector.tensor_tensor(mPm[:], psP[:], maskp_t[:], op=ALU.mult)
            ptr_cols = sm.tile([P, S], f32, tag="ptr_cols")
            nc.vector.tensor_reduce(
                ptr_cols[:],
                mPm[:].rearrange("p (g j) -> p g j", j=G),
                axis=mybir.AxisListType.X,
                op=ALU.add,
            )

            # ---- transposes (mT), W1T = A.T @ mT, VT = Cv.T @ mT ----
            mTt = big.tile([P, S * F], f32r, tag="mT")
            w1T = big.tile([P, S * F], f32r, tag="w1T")
            vta = big.tile([P, S * F], bf16, tag="vta")
            for sb in range(4):
                cols = slice(512 * sb, 512 * (sb + 1))
                psT = ps_big.tile([P, 512], f32, tag="ps512")
                for j in range(4):
                    nc.tensor.matmul(
                        psT[:, 128 * j : 128 * (j + 1)],
                        mn[:, 512 * sb + 128 * j : 512 * sb + 128 * (j + 1)],
                        ident_t[:],
                        is_transpose=True,
                        start=(j == 0),
                        stop=(j == 3),
                    )
                # scatter into b-outer layout: col = 16*b + s  (s = 4*sb + j)
                mT_dst = mTt[:].rearrange("p (b s) -> p s b", s=S)[
                    :, 4 * sb : 4 * (sb + 1), :
                ]
                if sb % 2 == 0:
                    nc.vector.tensor_copy(mT_dst, psT[:])
                else:
                    nc.scalar.copy(mT_dst, psT[:])

            for sb in range(4):
                cols = slice(512 * sb, 512 * (sb + 1))
                psW = ps_big.tile([P, 512], f32, tag="ps512")
                nc.tensor.matmul(
                    psW[:], A_t[:], mTt[:, cols]
                )
                if sb % 2 == 0:
                    nc.scalar.copy(w1T[:, cols], psW[:])
                else:
                    nc.vector.tensor_copy(w1T[:, cols], psW[:])

                psV = ps_big.tile([P, 512], f32, tag="ps512")
                nc.tensor.matmul(
                    psV[:], Cv_t[:], mTt[:, cols]
                )
                nc.vector.tensor_copy(vta[:, cols], psV[:])

            # ---- scores + softmax per group of 8 samples ----
            attn_all = big.tile([P, S * F], bf16, tag="attn_all")
            vsm_all = big.tile([P, S * F], bf16, tag="vsm_all")
            nc.sync.dma_start_transpose(
                vsm_all[:].rearrange("p (g q) -> p g q", q=P), vta[:]
            )
            negmax = sm.tile([P, S], f32, tag="negmax")
            rsums = sm.tile([P, S], f32, tag="rsums")

            for g in range(S):
                k = g // 2
                half = g % 2
                blkr_t = blkre_t if half == 0 else blkro_t
                ps_sc_t = ps_sc.tile([P, 256], f32, tag="sc")
                # mask first (-1e30 everywhere, +1e30 on own diag blocks),
                # then accumulate scores: off-diag stays -1e30, diag exact.
                nc.tensor.matmul(
                    ps_sc_t[:], mneg_t[:], mone_t[:], start=True, stop=False
                )
                nc.tensor.matmul(
                    ps_sc_t[:], blkl_t[:], blkr_t[:], start=False, stop=False
                )
                nc.tensor.matmul(
                    ps_sc_t[:],
                    w1T[:, 128 * g : 128 * (g + 1)],
                    mTt[:, 256 * k : 256 * (k + 1)],
                    start=False,
                    stop=True,
                )
                half_ap = ps_sc_t[:, 128 * half : 128 * (half + 1)]
                nc.vector.tensor_reduce(
                    negmax[:, g : g + 1], half_ap,
                    axis=mybir.AxisListType.X, op=ALU.max, negate=True,
                )
                attn_g = attn_all[:, 128 * g : 128 * (g + 1)]
                nc.scalar.activation(
                    attn_g,
                    half_ap,
                    AF.Exp,
                    bias=negmax[:, g : g + 1],
                    scale=1.0,
                    accum_out=rsums[:, g : g + 1],
                )

            recip = sm.tile([P, S], f32, tag="recip")
            nc.vector.reciprocal(recip[:], rsums[:])
            wS = sm.tile([P, S], f32, tag="wS")
            nc.vector.tensor_tensor(wS[:], recip[:], ptr_cols[:], op=ALU.mult)

            # ---- read, folded through attention ----
            # z_read_g = W8_g^T attn_g vsm_g, W8_g = onesblk * wS[:, g]
            # Y_g = attn_g^T @ W8_g  [ (b,t), j ];  z_read_g = Y_g^T @ vsm_g [j, o]
            zr = sm.tile([G, S * F], f32, tag="zr")
            W8a = sm.tile([P, G * S], bf16, tag="w8")
            for g in range(S):
                nc.gpsimd.tensor_scalar_mul(
                    W8a[:, G * g : G * (g + 1)], onesb_t[:], wS[:, g : g + 1]
                )
            for g in range(S):
                q = g % 4
                if q == 0:
                    psY4 = ps_sc.tile([P, 4 * G], f32, tag="sc")
                nc.tensor.matmul(
                    psY4[:, G * q : G * (q + 1)],
                    attn_all[:, 128 * g : 128 * (g + 1)],
                    W8a[:, G * g : G * (g + 1)],
                    start=(q == 0),
                    stop=(q == 3),
                )
                if q == 3:
                    Ysb4 = grp.tile([P, 4 * G], bf16, tag="Ysb")
                    nc.scalar.copy(Ysb4[:], psY4[:])
                    for q2 in range(4):
                        g2 = (g // 4) * 4 + q2
                        if q2 == 0:
                            ps_zr = ps_zrp.tile([G, 512], f32, tag="zrq")
                        nc.tensor.matmul(
                            ps_zr[:, 128 * q2 : 128 * (q2 + 1)],
                            Ysb4[:, G * q2 : G * (q2 + 1)],
                            vsm_all[:, 128 * g2 : 128 * (g2 + 1)],
                            start=(q2 == 0),
                            stop=(q2 == 3),
                        )
                    c = g // 4
                    nc.scalar.copy(zr[:, 512 * c : 512 * (c + 1)], ps_zr[:])
            zr3 = zr[:].rearrange("p (g o) -> p g o", o=F)
            nc.sync.dma_start(
                zrr_h[rows].rearrange("(g b) o -> b g o", b=G), zr3[:, :, 0:D]
            )
            nc.sync.dma_start(
                zri_h[rows].rearrange("(g b) o -> b g o", b=G), zr3[:, :, D:F]
            )

        # ---- active_slots: cross-partition sum of acc ----
        psA = ps_zrp.tile([1, 1], f32, tag="zrq")
        nc.tensor.matmul(psA[:], acc_t[:], ones_t[:])
        act_s = sm.tile([1, 1], f32, tag="act_s")
        nc.vector.tensor_copy(act_s[:], psA[:])
        nc.sync.dma_start(act_h[:], act_s[:])

    nc.compile()
    return nc


# ----------------------------------------------------------------------------
# host-side constants
# ----------------------------------------------------------------------------


def host_constants(w_q_real, w_q_imag, w_k_real, w_k_imag, w_v_real, w_v_imag):
    def cmat(wr, wi):
        wr = np.asarray(wr, np.float64)
        wi = np.asarray(wi, np.float64)
        top = np.concatenate([wr.T, wi.T], axis=1)
        bot = np.concatenate([-wi.T, wr.T], axis=1)
        return np.concatenate([top, bot], axis=0)  # [2D, 2D]

    Cq = cmat(w_q_real, w_q_imag)
    Ck = cmat(w_k_real, w_k_imag)
    Cv = cmat(w_v_real, w_v_imag)
    A = (Cq @ Ck.T) * (D ** -0.5)

    ident = np.eye(P, dtype=np.float32)
    onesblk = np.zeros((P, G), np.float32)
    for p_ in range(P):
        onesblk[p_, p_ // S] = 1.0
    maskP = np.zeros((P, P), np.float32)
    for p_ in range(P):
        b = p_ // S
        for col in range(P):
            if col % G == b:
                maskP[p_, col] = 1.0
    import ml_dtypes

    BIG = 1.0e30
    mneg1 = np.full((1, P), -BIG, np.float32)
    mone1 = np.ones((1, 2 * P), np.float32)
    blkL = np.zeros((G, P), np.float32)
    for j in range(G):
        for p_ in range(P):
            if p_ // S == j:
                blkL[j, p_] = BIG
    blkRe = np.zeros((G, 2 * P), np.float32)
    blkRo = np.zeros((G, 2 * P), np.float32)
    for j in range(G):
        for n in range(2 * P):
            bp = n // S  # sample index within the 16-sample pair
            if bp == j:
                blkRe[j, n] = 1.0
            if bp == G + j:
                blkRo[j, n] = 1.0

    return {
        "Amat": A.astype(np.float32),
        "Cv": Cv.astype(np.float32),
        "ident": ident,
        "onesblk": onesblk.astype(ml_dtypes.bfloat16),
        "maskP": maskP,
        "mneg1": mneg1.astype(ml_dtypes.bfloat16),
        "mone1": mone1.astype(ml_dtypes.bfloat16),
        "blkL": blkL.astype(ml_dtypes.bfloat16),
        "blkRe": blkRe.astype(ml_dtypes.bfloat16),
        "blkRo": blkRo.astype(ml_dtypes.bfloat16),
    }


def make_in_maps(inputs, b_core):
    consts = host_constants(
        inputs["w_q_real"], inputs["w_q_imag"],
        inputs["w_k_real"], inputs["w_k_imag"],
        inputs["w_v_real"], inputs["w_v_imag"],
    )
    n_cores = int(np.asarray(inputs["mem"]).shape[0]) // b_core
    in_maps = []
    for c in range(n_cores):
        rows = slice(c * b_core, (c + 1) * b_core)
        m = {
            "mem": np.ascontiguousarray(np.asarray(inputs["mem"], np.float32)[rows]),
            "z_real": np.ascontiguousarray(np.asarray(inputs["z_real"], np.float32)[rows]),
            "z_imag": np.ascontiguousarray(np.asarray(inputs["z_imag"], np.float32)[rows]),
            "ptr": np.ascontiguousarray(np.asarray(inputs["ptr"], np.float32)[rows]),
            "ctrl": np.ascontiguousarray(np.asarray(inputs["ctrl"], np.float32)[rows]),
        }
        m.update(consts)
        in_maps.append(m)
    return in_maps


_PROGRAM_CACHE = {}


def _get_program(b_core):
    if b_core not in _PROGRAM_CACHE:
        _PROGRAM_CACHE[b_core] = build_program(b_core)
    return _PROGRAM_CACHE[b_core]


def kernel(**inputs):
    B = int(np.asarray(inputs["z_real"]).shape[0])
    b_core = B // N_CORES
    nc = _get_program(b_core)
    in_maps = make_in_maps(inputs, b_core)

    from concourse.bass_utils import run_bass_kernel_spmd

    try:
        res = run_bass_kernel_spmd(nc, in_maps, list(range(N_CORES)))
    except Exception:
        # transient axon/NRT device-state failures self-heal; retry once
        import time as _time

        _time.sleep(20)
        res = run_bass_kernel_spmd(nc, in_maps, list(range(N_CORES)))
    outs = res.results

    z_read_real = np.concatenate([o["z_read_real"] for o in outs], axis=0)
    z_read_imag = np.concatenate([o["z_read_imag"] for o in outs], axis=0)
    mem_new = np.concatenate([o["mem_new"] for o in outs], axis=0)
    new_ptr = np.concatenate([o["new_ptr"] for o in outs], axis=0)
    active = np.float32(sum(float(o["active_part"][0, 0]) for o in outs) / B)
    return (
        z_read_real.astype(np.float32),
        z_read_imag.astype(np.float32),
        mem_new.astype(np.float32),
        new_ptr.astype(np.float32),
        np.float32(active),
    )
